# revision 7
# baseline (speedup 1.0000x reference)
"""DGCNN (DynamicEdgeConv x3 + MLP head) Trainium2 Bass kernel.

Data-parallel over the 16 point clouds: 8 NeuronCores x 2 clouds each.
Everything for one cloud runs on one core; parameters replicated.

Per cloud (P=2048 points):
  kNN ranking value  val[p,q] = <f_p,f_q> - |f_q|^2/2  (order-equivalent to
  -dist/2) computed by PE matmuls with an augmented contraction row
  (ones / -|f|^2/2), top-3 per row via DVE max8 + max_index (self is always
  rank-1), neighbor indices rearranged into GPSIMD ap_gather's 16-wrapped
  layout by DVE 32x32 block transposes, neighbor features gathered on
  GPSIMD.  EdgeConv MLP layer-1 is split U_i + V_j, BN affines are folded
  into following weights where legal (max commutes with the monotonic
  relu/affine, gamma>0).  All matmuls run in fp32r (TF32-like).
"""

import sys
from contextlib import ExitStack

import numpy as np

sys.path.insert(0, "/opt/trn_rl_repo")

import concourse.bass as bass  # noqa: F401
import concourse.mybir as mybir
import concourse.tile as tile
from concourse import bacc

B, P, K = 16, 2048, 3
N = B * P
OUT = 13
BN_EPS = 1e-5
N_CORES = 8
CPC = 2  # clouds per core
dt = mybir.dt
AF = mybir.ActivationFunctionType
ALU = mybir.AluOpType

_CACHE = {}


# ---------------------------------------------------------------- host utils
def _rnd12(x):
    m, e = np.frexp(np.asarray(x, np.float32))
    return np.ldexp(np.round(m * 4096) / 4096, e).astype(np.float32)


def _np(v):
    return np.asarray(v, dtype=np.float32)


def _prep_weights(params):
    """Fold BN affines; split edge-conv layer 1; build device layouts."""
    inv = float(1.0 / np.sqrt(1.0 + BN_EPS))
    w = {}
    ccols = []
    for ci, cname in enumerate(("c1", "c2", "c3")):
        l1, l2 = params[cname]
        W1, b1 = _np(l1["W"]), _np(l1["b"])
        s1, t1 = _np(l1["g"]) * inv, _np(l1["be"])
        W2, b2 = _np(l2["W"]), _np(l2["b"])
        s2, t2 = _np(l2["g"]) * inv, _np(l2["be"])
        D = W1.shape[0] // 2
        Wd = W1[:D] - W1[D:]
        Wv = W1[D:]
        W2p = s1[:, None] * W2
        b2p = t1 @ W2 + b2
        w[f"wuv{ci + 1}"] = np.concatenate([Wd, Wv], axis=1)  # [D, 128]
        w[f"w2p{ci + 1}"] = W2p  # [64, 64]
        ccols += [b1, s2, s2 * b2p, t2]
    w["cconst"] = np.stack(ccols, axis=1)  # [64, 12]

    l1 = params["lin1"][0]
    w["wl1"] = _np(l1["W"])  # [192, 1024]
    w["bl1"] = _np(l1["b"])
    s_l1, t_l1 = _np(l1["g"]) * inv, _np(l1["be"])
    m1 = params["m1"][0]
    w["wm1"] = s_l1[:, None] * _np(m1["W"])  # [1024, 256]
    w["bm1"] = t_l1 @ _np(m1["W"]) + _np(m1["b"])
    s_m1, t_m1 = _np(m1["g"]) * inv, _np(m1["be"])
    m2 = params["m2"][0]
    w["wm2"] = s_m1[:, None] * _np(m2["W"])  # [256, 128]
    w["bm2"] = t_m1 @ _np(m2["W"]) + _np(m2["b"])
    s_m2, t_m2 = _np(m2["g"]) * inv, _np(m2["be"])
    fin = params["fin"]
    wf = s_m2[:, None] * _np(fin["W"])  # [128, 13]
    w["wfin"] = np.concatenate([wf, np.zeros((128, 3), np.float32)], axis=1)
    bfin = t_m2 @ _np(fin["W"]) + _np(fin["b"])
    w["bfin"] = np.broadcast_to(bfin, (128, OUT)).copy()
    w["neghalf"] = np.full((64, 1), -0.5, np.float32)
    w["onesrow"] = np.ones((1, P), np.float32)
    return w


# ---------------------------------------------------------------- bass build
def _build_nc():
    nc = bacc.Bacc("TRN2", target_bir_lowering=False, debug=False)

    def din(name, shape, d=dt.float32r):
        return nc.dram_tensor(name, shape, d, kind="ExternalInput").ap()

    dd = {}
    dd["aL1"] = [din(f"aL1{c}", [7, P]) for c in range(CPC)]
    dd["aR1"] = [din(f"aR1{c}", [7, P]) for c in range(CPC)]
    dd["wuv"] = [din("wuv1", [6, 128]), din("wuv2", [64, 128]), din("wuv3", [64, 128])]
    dd["w2p"] = [din(f"w2p{i}", [64, 64]) for i in (1, 2, 3)]
    dd["wl1"] = din("wl1", [192, 1024])
    dd["wm1"] = din("wm1", [1024, 256])
    dd["wm2"] = din("wm2", [256, 128])
    dd["wfin"] = din("wfin", [128, 16])
    dd["bl1"] = din("bl1", [1024], dt.float32)
    dd["bm1"] = din("bm1", [256], dt.float32)
    dd["bm2"] = din("bm2", [128], dt.float32)
    dd["bfin"] = din("bfin", [128, OUT], dt.float32)
    dd["cconst"] = din("cconst", [64, 12], dt.float32)
    dd["neghalf"] = din("neghalf", [64, 1])
    dd["onesrow"] = din("onesrow", [1, P])
    dd["out"] = nc.dram_tensor("o", [CPC * P, OUT], dt.float32, kind="ExternalOutput").ap()

    with tile.TileContext(nc) as tc:
        _body(nc, tc, dd)
    nc.compile()
    return nc


def _body(nc, tc, dd):
    NT = P // 128  # 16 point tiles per cloud
    NC4 = P // 512  # 4 col chunks

    with ExitStack() as ctx:
        persist = ctx.enter_context(tc.tile_pool(name="persist", bufs=1))
        work = ctx.enter_context(tc.tile_pool(name="work", bufs=1))
        v8pool = ctx.enter_context(tc.tile_pool(name="v8", bufs=4))
        augpool = ctx.enter_context(tc.tile_pool(name="augp", bufs=1))
        chunks = ctx.enter_context(tc.tile_pool(name="chunks", bufs=1))

        # ---- weights/consts
        wuv, w2p = [], []
        for i in range(3):
            t = persist.tile([6 if i == 0 else 64, 128], dt.float32r, tag=f"wuv{i}")
            nc.sync.dma_start(t[:], dd["wuv"][i])
            wuv.append(t)
            t = persist.tile([64, 64], dt.float32r, tag=f"w2p{i}")
            nc.sync.dma_start(t[:], dd["w2p"][i])
            w2p.append(t)
        wl1 = persist.tile([64, 3, 1024], dt.float32r, tag="wl1")
        nc.sync.dma_start(wl1[:], dd["wl1"].rearrange("(kc p) m -> p kc m", p=64))
        wm1 = persist.tile([128, 8, 256], dt.float32r, tag="wm1")
        nc.sync.dma_start(wm1[:], dd["wm1"].rearrange("(kt p) m -> p kt m", p=128))
        wm2 = persist.tile([128, 2, 128], dt.float32r, tag="wm2")
        nc.sync.dma_start(wm2[:], dd["wm2"].rearrange("(kt p) m -> p kt m", p=128))
        wfin = persist.tile([128, 16], dt.float32r, tag="wfin")
        nc.sync.dma_start(wfin[:], dd["wfin"])
        bl1 = persist.tile([128, 8], dt.float32, tag="bl1")
        nc.sync.dma_start(bl1[:], dd["bl1"].rearrange("(mt p) -> p mt", p=128))
        bm1 = persist.tile([128, 2], dt.float32, tag="bm1")
        nc.sync.dma_start(bm1[:], dd["bm1"].rearrange("(mt p) -> p mt", p=128))
        bm2 = persist.tile([128, 1], dt.float32, tag="bm2")
        nc.sync.dma_start(bm2[:], dd["bm2"].rearrange("(mt p) -> p mt", p=128))
        bfin = persist.tile([128, OUT], dt.float32, tag="bfin")
        nc.sync.dma_start(bfin[:], dd["bfin"])
        cconst = persist.tile([64, 12], dt.float32, tag="cconst")
        nc.sync.dma_start(cconst[:], dd["cconst"])
        neghalf = persist.tile([64, 1], dt.float32r, tag="neghalf")
        nc.sync.dma_start(neghalf[:], dd["neghalf"])

        # persistent feature tiles (consumed by the MLP head)
        x1T = [persist.tile([65, P], dt.float32r, tag=f"x1T{cl}", name=f"x1T{cl}") for cl in range(CPC)]
        x2T = [persist.tile([65, P], dt.float32r, tag=f"x2T{cl}", name=f"x2T{cl}") for cl in range(CPC)]
        x3T = [persist.tile([64, P], dt.float32r, tag=f"x3T{cl}", name=f"x3T{cl}") for cl in range(CPC)]
        for cl in range(CPC):
            nc.sync.dma_start(x1T[cl][64:65, :], dd["onesrow"])
            nc.sync.dma_start(x2T[cl][64:65, :], dd["onesrow"])
        L = [persist.tile([128, 16, OUT], dt.float32, tag=f"L{cl}", name=f"L{cl}") for cl in range(CPC)]

        # augR tiles are cycled per cloud via a shared tag
        augR_cur = [None, None]

        # ====================================================== edge convs
        for conv in range(3):
            D = 6 if conv == 0 else 64
            b1c = cconst[:, 4 * conv : 4 * conv + 1]
            s2c = cconst[:, 4 * conv + 1 : 4 * conv + 2]
            s2b2c = cconst[:, 4 * conv + 2 : 4 * conv + 3]
            t2c = cconst[:, 4 * conv + 3 : 4 * conv + 4]

            for cl in range(CPC):
                # ---- feature tiles for this conv
                if conv == 0:
                    aL = augpool.tile([7, P], dt.float32r, tag="augL1")
                    nc.sync.dma_start(aL[:], dd["aL1"][cl])
                    aR = augpool.tile([7, P], dt.float32r, tag=f"augR{cl}")
                    nc.sync.dma_start(aR[:], dd["aR1"][cl])
                else:
                    aL = (x1T, x2T)[conv - 1][cl]
                    aR = augR_cur[cl]

                # ---- phase A: sq row (conv>0) + U/V
                U = work.tile([64, P], dt.float32, tag="U")
                V = work.tile([64, P], dt.float32, tag="V")
                with tc.tile_pool(name=f"psA{conv}{cl}", bufs=2, space="PSUM") as psA:
                    if conv > 0:
                        f2 = work.tile([64, P], dt.float32r, tag="h1s")
                        nc.vector.tensor_mul(
                            f2[:],
                            aL[0:64, :].bitcast(dt.float32),
                            aR[0:64, :].bitcast(dt.float32),
                        )
                        for c in range(NC4):
                            cs = slice(512 * c, 512 * (c + 1))
                            sqp = psA.tile([1, 512], dt.float32, tag="sq")
                            nc.tensor.matmul(
                                sqp[:], neghalf[:], f2[:, cs], start=True, stop=True
                            )
                            nc.scalar.copy(aR[64:65, cs], sqp[:])
                    for c in range(NC4):
                        cs = slice(512 * c, 512 * (c + 1))
                        up = psA.tile([64, 512], dt.float32, tag="uv")
                        nc.tensor.matmul(
                            up[:], wuv[conv][0:D, 0:64], aL[0:D, cs],
                            start=True, stop=True,
                        )
                        nc.scalar.copy(U[:, cs], up[:])
                        vp = psA.tile([64, 512], dt.float32, tag="uv")
                        nc.tensor.matmul(
                            vp[:], wuv[conv][0:D, 64:128], aL[0:D, cs],
                            start=True, stop=True,
                        )
                        nc.scalar.copy(V[:, cs], vp[:])

                # ---- phase B: kNN top-8 values + indices per point tile
                X = work.tile([128, 128], dt.uint32, tag="X")
                X3 = X[:].rearrange("p (s t) -> p s t", t=16)
                with tc.tile_pool(name=f"psG{conv}{cl}", bufs=2, space="PSUM") as psG:
                    for t in range(NT):
                        gp = psG.tile([128, P], dt.float32, tag="G")
                        for c in range(NC4):
                            nc.tensor.matmul(
                                gp[:, 512 * c : 512 * (c + 1)],
                                aL[0 : D + 1, 128 * t : 128 * (t + 1)],
                                aR[0 : D + 1, 512 * c : 512 * (c + 1)],
                                start=True,
                                stop=True,
                            )
                        v8 = v8pool.tile([128, 8], dt.float32, tag="v8")
                        nc.vector.max(out=v8[:], in_=gp[:])
                        nc.vector.max_index(
                            out=X3[:, :, t], in_max=v8[:], in_values=gp[:]
                        )

                # ---- phase C: gather neighbors, edge MLP, max-k, output
                if conv < 2:
                    nL = (x1T, x2T)[conv][cl]
                    nR = augpool.tile([65, P], dt.float32r, tag=f"augR{cl}")
                    augR_cur[cl] = nR
                else:
                    nL = x3T[cl]

                T0 = work.tile([32, 128], dt.uint32, tag="T0")
                T1 = work.tile([32, 128], dt.uint32, tag="T1")
                for b in range(4):
                    bs = slice(32 * b, 32 * (b + 1))
                    nc.vector.transpose(out=T0[:, bs], in_=X[bs, 0:32])
                    nc.vector.transpose(out=T1[:, bs], in_=X[bs, 32:64])
                T0c = work.tile([32, 128], dt.int16, tag="T0c")
                T1c = work.tile([32, 128], dt.int16, tag="T1c")
                nc.vector.tensor_copy(T0c[:], T0[:])
                nc.vector.tensor_copy(T1c[:], T1[:])
                idxg = work.tile([64, 256], dt.int16, tag="idxg")
                for g in range(4):
                    gs = slice(16 * g, 16 * (g + 1))
                    nc.sync.dma_start(idxg[gs, 0:128], T0c[16:32, :])
                    nc.sync.dma_start(idxg[gs, 128:256], T1c[0:16, :])
                vg = work.tile([64, 4096], dt.float32, tag="vg")
                nc.gpsimd.ap_gather(
                    out_ap=vg[:, :, None],
                    in_ap=V[:, :, None],
                    idxs_ap=idxg[:],
                    channels=64,
                    num_elems=P,
                    d=1,
                    num_idxs=4096,
                )
                E = work.tile([64, 3 * P], dt.float32r, tag="E")
                Upv = U[:].rearrange("c (q h) -> c h q", q=16)
                h1s = work.tile([64, P], dt.float32, tag="h1s")
                h13 = h1s[:].rearrange("p (h q) -> p h q", q=16)
                for k in range(3):
                    if k == 0:
                        vin = V[:].rearrange("c (q h) -> c h q", q=16)
                    else:
                        vin = vg[:, 2048 * (k - 1) : 2048 * k].rearrange(
                            "p (h q) -> p h q", q=16
                        )
                    nc.vector.scalar_tensor_tensor(
                        out=h13, in0=Upv, scalar=b1c, in1=vin,
                        op0=ALU.add, op1=ALU.add,
                    )
                    nc.scalar.activation(
                        E[:, 2048 * k : 2048 * (k + 1)], h1s[:], AF.Relu
                    )
                with tc.tile_pool(name=f"psE{conv}{cl}", bufs=6, space="PSUM") as psE:
                    for c in range(NC4):
                        hp = []
                        for k in range(3):
                            p = psE.tile([64, 512], dt.float32, tag="h2")
                            nc.tensor.matmul(
                                p[:],
                                w2p[conv][:],
                                E[:, 2048 * k + 512 * c : 2048 * k + 512 * (c + 1)],
                                start=True,
                                stop=True,
                            )
                            hp.append(p)
                        mx = work.tile([64, 512], dt.float32, tag="mx")
                        nc.scalar.copy(mx[:], hp[0][:])
                        nc.vector.tensor_tensor(mx[:], mx[:], hp[1][:], ALU.max)
                        nc.vector.tensor_tensor(mx[:], mx[:], hp[2][:], ALU.max)
                        xnx = work.tile([64, 512], dt.float32, tag="xnx")
                        nc.scalar.activation(
                            xnx[:], mx[:], AF.Relu, bias=s2b2c, scale=s2c
                        )
                        dst = nL[0:64, :].rearrange("c (q h) -> c h q", q=16)[
                            :, 32 * c : 32 * (c + 1), :
                        ]
                        nc.vector.tensor_scalar_add(
                            dst, xnx[:].rearrange("c (h q) -> c h q", q=16), t2c
                        )
                if conv < 2:
                    nc.scalar.copy(nR[0:64, :], nL[0:64, :])

        # ====================================================== MLP head
        with tc.tile_pool(name="psD", bufs=2, space="PSUM") as psD:
            for cl in range(CPC):
                xs = (x1T[cl], x2T[cl], x3T[cl])
                for oc in range(NC4):
                    cs = slice(512 * oc, 512 * (oc + 1))
                    a1 = chunks.tile([128, 8, 512], dt.float32r, tag="a1")
                    for mt in range(8):
                        pp = psD.tile([128, 512], dt.float32, tag="l1")
                        for kc in range(3):
                            nc.tensor.matmul(
                                pp[:],
                                wl1[:, kc, 128 * mt : 128 * (mt + 1)],
                                xs[kc][0:64, cs],
                                start=(kc == 0),
                                stop=(kc == 2),
                            )
                        nc.scalar.activation(
                            a1[:, mt, :], pp[:], AF.Relu, bias=bl1[:, mt : mt + 1]
                        )
                    a2 = chunks.tile([128, 2, 512], dt.float32r, tag="a2")
                    for mt in range(2):
                        pp = psD.tile([128, 512], dt.float32, tag="m1")
                        for kt in range(8):
                            nc.tensor.matmul(
                                pp[:],
                                wm1[:, kt, 128 * mt : 128 * (mt + 1)],
                                a1[:, kt, :],
                                start=(kt == 0),
                                stop=(kt == 7),
                            )
                        nc.scalar.activation(
                            a2[:, mt, :], pp[:], AF.Relu, bias=bm1[:, mt : mt + 1]
                        )
                    a3 = chunks.tile([128, 512], dt.float32r, tag="a3")
                    pp = psD.tile([128, 512], dt.float32, tag="m2")
                    for kt in range(2):
                        nc.tensor.matmul(
                            pp[:],
                            wm2[:, kt, :],
                            a2[:, kt, :],
                            start=(kt == 0),
                            stop=(kt == 1),
                        )
                    nc.scalar.activation(a3[:], pp[:], AF.Relu, bias=bm2[:, 0:1])
                    for s in range(4):
                        pf = psD.tile([128, 16], dt.float32, tag="fin")
                        nc.tensor.matmul(
                            pf[:],
                            a3[:, 128 * s : 128 * (s + 1)],
                            wfin[:],
                            start=True,
                            stop=True,
                        )
                        nc.vector.tensor_add(L[cl][:, 4 * oc + s, :], pf[:, 0:OUT], bfin[:])

        # log-softmax over the 13 classes + output DMA
        for cl in range(CPC):
            L3 = L[cl][:]
            rmax = work.tile([128, 16], dt.float32, tag="rmax")
            nc.vector.tensor_reduce(
                out=rmax[:], in_=L3, axis=mybir.AxisListType.X, op=ALU.max
            )
            tt = work.tile([128, 16, OUT], dt.float32, tag="tt")
            nc.vector.tensor_tensor(
                tt[:], L3, rmax[:, :, None].to_broadcast([128, 16, OUT]), ALU.subtract
            )
            ee = work.tile([128, 16, OUT], dt.float32, tag="mx")
            nc.scalar.activation(ee[:], tt[:], AF.Exp)
            ssum = work.tile([128, 16], dt.float32, tag="ssum")
            nc.vector.tensor_reduce(
                out=ssum[:], in_=ee[:], axis=mybir.AxisListType.X, op=ALU.add
            )
            lse = work.tile([128, 16], dt.float32, tag="lse")
            nc.scalar.activation(lse[:], ssum[:], AF.Ln)
            oo = work.tile([128, 16, OUT], dt.float32, tag="xnx")
            nc.vector.tensor_tensor(
                oo[:], tt[:], lse[:, :, None].to_broadcast([128, 16, OUT]),
                ALU.subtract,
            )
            nc.sync.dma_start(
                dd["out"][P * cl : P * (cl + 1), :].rearrange("(t p) c -> p t c", p=128),
                oo[:],
            )


# ---------------------------------------------------------------- execution
class _Runner:
    def __init__(self, nc, n_cores):
        import jax
        from jax.sharding import Mesh, PartitionSpec
        from jax.experimental.shard_map import shard_map
        from concourse.bass2jax import (
            _bass_exec_p,
            install_neuronx_cc_hook,
            partition_id_tensor,
        )

        install_neuronx_cc_hook()
        self.jax = jax
        self.n_cores = n_cores
        pname = nc.partition_id_tensor.name if nc.partition_id_tensor else None
        in_names, out_names, out_avals, zero_outs = [], [], [], []
        for alloc in nc.m.functions[0].allocations:
            if not isinstance(alloc, mybir.MemoryLocationSet):
                continue
            name = alloc.memorylocations[0].name
            if alloc.kind == "ExternalInput":
                if name != pname:
                    in_names.append(name)
            elif alloc.kind == "ExternalOutput":
                out_names.append(name)
                shape = tuple(alloc.tensor_shape)
                dtp = mybir.dt.np(alloc.dtype)
                out_avals.append(jax.core.ShapedArray(shape, dtp))
                zero_outs.append(np.zeros(shape, dtp))
        self.in_names, self.out_names = in_names, out_names
        self.out_avals, self.zero_outs = out_avals, zero_outs
        n_params, n_outs = len(in_names), len(out_names)
        all_in = list(in_names) + list(out_names)
        if pname is not None:
            all_in.append(pname)

        def _b(*args):
            ops = list(args)
            if pname is not None:
                ops.append(partition_id_tensor())
            return tuple(
                _bass_exec_p.bind(
                    *ops,
                    out_avals=tuple(out_avals),
                    in_names=tuple(all_in),
                    out_names=tuple(out_names),
                    lowering_input_output_aliases=(),
                    sim_require_finite=True,
                    sim_require_nnan=True,
                    nc=nc,
                )
            )

        devices = jax.devices()[:n_cores]
        mesh = Mesh(np.asarray(devices), ("core",))
        specs = (PartitionSpec("core"),) * (n_params + n_outs)
        self._fn = jax.jit(
            shard_map(
                _b,
                mesh=mesh,
                in_specs=specs,
                out_specs=(PartitionSpec("core"),) * n_outs,
                check_rep=False,
            ),
            keep_unused=True,
        )

    def run(self, in_maps):
        args = [
            np.concatenate([np.ascontiguousarray(m[n]) for m in in_maps], axis=0)
            for n in self.in_names
        ] + [
            np.zeros((self.n_cores * z.shape[0], *z.shape[1:]), z.dtype)
            for z in self.zero_outs
        ]
        outs = self._fn(*args)
        self.jax.block_until_ready(outs)
        return [
            {
                n: np.asarray(outs[i]).reshape(self.n_cores, *self.out_avals[i].shape)[c]
                for i, n in enumerate(self.out_names)
            }
            for c in range(self.n_cores)
        ]


def _get_runner():
    if "runner" not in _CACHE:
        nc = _build_nc()
        _CACHE["runner"] = _Runner(nc, N_CORES)
    return _CACHE["runner"]


def _make_in_maps(x, pos, w):
    in_maps = []
    for core in range(N_CORES):
        m = dict(w)
        for cl in range(CPC):
            ci = core * CPC + cl
            x0 = np.concatenate(
                [x[ci * P : (ci + 1) * P], pos[ci * P : (ci + 1) * P]], axis=1
            )
            x0T = np.ascontiguousarray(x0.T, dtype=np.float32)  # [6, P]
            sqh = 0.5 * (_rnd12(x0T) ** 2).sum(0)
            m[f"aL1{cl}"] = np.concatenate([x0T, np.ones((1, P), np.float32)], axis=0)
            m[f"aR1{cl}"] = np.concatenate([x0T, -sqh[None, :]], axis=0)
        in_maps.append(m)
    return in_maps


def kernel(x, pos, batch, params):
    x = np.asarray(x, np.float32)
    pos = np.asarray(pos, np.float32)
    w = _prep_weights(params)
    runner = _get_runner()
    in_maps = _make_in_maps(x, pos, w)
    results = runner.run(in_maps)
    out = np.empty((N, OUT), np.float32)
    for core in range(N_CORES):
        out[core * CPC * P : (core + 1) * CPC * P] = results[core]["o"]
    return out
